# revision 2
# baseline (speedup 1.0000x reference)
"""Trainium2 Bass kernel for nn_Conv2dAMPS.

Reference computation: im2col with a 2x2 kernel (4 positions), per-sample
matrices M_w = tensors[w] . emb_w (contract channels), output = row 0 of
M_0 @ M_1 @ M_2 @ M_3, reshaped to (B, O, oh, ow).

Only row 0 of the matrix product is needed, so the chain collapses to a
vector-matrix chain per sample:
    v0 = A0 @ emb0                (A0[p,j] = tensors[0,0,j,p])
    v_k[j] = sum_{i,p} v_{k-1}[i] * emb_k[p] * T_k[i,j,p],  k = 1,2,3
Each step is one 4096-contraction matmul whose rhs z = v_{k-1} (x) emb_k
(per-sample outer product) is built on the vector engine from
partition-replicated operand tiles produced by 0/1-selection matmuls on the
tensor engine.  Chunks of the 4096 axis are 8 i's x 16 p's; even/odd chunks
run concurrently in the PE array via column tiling (top/bottom halves of one
PSUM tile), and the top+bottom fold is absorbed into the next step's
replication matmul (K=128 selection).

im2col: emb_1[n] = x[oy, ox+1], emb_2[n] = x[oy+1, ox] = e0ext[n+31],
emb_3[n] = x[oy+1, ox+1] = e1ext[n+31] -- so only two extended (32-row)
image loads are needed; the +31 shifts are applied when building the
replication patterns (PE-side, no alignment constraints).

Sharding: data-parallel over batch B (2 images per core, 8 cores), weights
replicated.
"""

import sys

sys.path.insert(0, "/opt/trn_rl_repo")

import numpy as np
import ml_dtypes

import concourse.bacc as bacc
import concourse.mybir as mybir
import concourse.tile as tile
from concourse import bass_utils

BF16 = ml_dtypes.bfloat16

B, C, H, W = 16, 64, 32, 32
O = 64
OH = OW = 31
NIMG = OH * OW            # 961 real samples per image
NEXT = 992                # extended im2col columns (32 rows x 31)
BLK = 1024                # column block per image
NCORES = 8
IPC = B // NCORES         # images per core
FD = IPC * BLK            # 2048 free columns per core
NQ = FD // 512            # psum quarters

A_SET = 8                 # i's per u-set
B_SET = 16                # p's per w-set
N_S = O // A_SET          # 8
N_T = O // B_SET          # 4
N_CHUNK = N_S * N_T       # 32

GP_CHUNK_MOD = 5   # chunks with c % 5 == 2 go to gpsimd (6 of 32 per step)

# (pattern source slot, source shift) per chain step k=1,2,3
STEP_SRC = {1: (1, 0), 2: (0, 31), 3: (1, 31)}


def _build_program(reps=1, loop_n=1):
    nc = bacc.Bacc("TRN2", target_bir_lowering=False, debug=False)
    dt = mybir.dt

    x_d = nc.dram_tensor("x", [IPC, C, H, W], dt.bfloat16,
                         kind="ExternalInput").ap()
    lhst_d = nc.dram_tensor("lhst", [3, N_CHUNK, 128, O], dt.bfloat16,
                            kind="ExternalInput").ap()
    a0_d = nc.dram_tensor("a0", [C, 128], dt.bfloat16, kind="ExternalInput").ap()
    r1_d = nc.dram_tensor("r1", [N_S, 128, 128], dt.bfloat16,
                          kind="ExternalInput").ap()
    r2_d = nc.dram_tensor("r2", [N_T, C, 128], dt.bfloat16,
                          kind="ExternalInput").ap()
    out_d = nc.dram_tensor("out", [IPC, O, NIMG], dt.float32,
                           kind="ExternalOutput").ap()

    with tile.TileContext(nc) as tc:
        with (
            tc.tile_pool(name="consts", bufs=1) as consts,
            tc.tile_pool(name="embp", bufs=1) as embp,
            tc.tile_pool(name="patp", bufs=1) as patp,
            tc.tile_pool(name="ops1", bufs=2) as ops1,
            tc.tile_pool(name="zp", bufs=6) as zp,
            tc.tile_pool(name="vp", bufs=2) as vp,
            tc.tile_pool(name="outp", bufs=1) as outp,
            tc.tile_pool(name="ps_op", bufs=2, space="PSUM") as ps_op,
            tc.tile_pool(name="ps_acc", bufs=1, space="PSUM") as ps_acc,
        ):
            # ---- constants ----
            lhst_sb = consts.tile([128, 3, N_CHUNK, O], dt.bfloat16)
            nc.sync.dma_start(out=lhst_sb, in_=lhst_d.rearrange("k c l j -> l k c j"))
            a0_sb = consts.tile([C, 128], dt.bfloat16)
            nc.sync.dma_start(out=a0_sb, in_=a0_d)
            r1_sb = consts.tile([128, N_S, 128], dt.bfloat16)
            nc.sync.dma_start(out=r1_sb, in_=r1_d.rearrange("s k l -> k s l"))
            r2_sb = consts.tile([C, N_T, 128], dt.bfloat16)
            nc.sync.dma_start(out=r2_sb, in_=r2_d.rearrange("t k l -> k t l"))

            loop_cm = tc.For_i(0, loop_n, 1) if loop_n > 1 else None
            import contextlib
            with (loop_cm if loop_cm is not None else contextlib.nullcontext()):
                for rep in range(reps):
                    # ---- extended im2col loads: e0ext = x[:, 0:32, 0:31], e1ext = x[:, 0:32, 1:32]
                    embT = embp.tile([C, 2, FD + 32], dt.bfloat16)
                    for b in range(IPC):
                        for e, dj in ((0, 0), (1, 1)):
                            dst = embT[:, e, b * BLK:b * BLK + NEXT]
                            dst = dst.rearrange("c (h w) -> c h w", h=H)
                            nc.sync.dma_start(out=dst, in_=x_d[b, :, 0:H, dj:dj + OW])

                    # ---- replication patterns: pat[k] used by step k's TT multiplies
                    pats = []
                    for k in (1, 2, 3):
                        e, off = STEP_SRC[k]
                        pat = patp.tile([128, N_T, FD], dt.bfloat16, tag=f"pat{k}",
                                        name=f"pat{rep}_{k}")
                        for t in range(N_T):
                            for hh in range(NQ // 2):
                                p2 = ps_op.tile([128, 1024], dt.float32, tag="op",
                                                name=f"patp_{rep}_{k}_{t}_{hh}")
                                for q in range(2):
                                    c0 = hh * 1024 + q * 512
                                    nc.tensor.matmul(p2[:, q * 512:(q + 1) * 512],
                                                     r2_sb[:, t, :],
                                                     embT[:, e, c0 + off:c0 + off + 512],
                                                     start=True, stop=True)
                                nc.scalar.copy(out=pat[:, t, hh * 1024:(hh + 1) * 1024],
                                               in_=p2)
                        pats.append(pat)

                    # ---- v0 ----
                    acc = [ps_acc.tile([128, 512], dt.float32, tag=f"acc{q}",
                                       name=f"acc_{rep}_{q}") for q in range(NQ)]
                    for q in range(NQ):
                        nc.tensor.matmul(acc[q], a0_sb,
                                         embT[:, 0, q * 512:(q + 1) * 512],
                                         start=True, stop=True)
                    vT = vp.tile([128, FD], dt.bfloat16, tag="v", name=f"v0_{rep}")
                    for q in range(NQ):
                        nc.scalar.copy(out=vT[:, q * 512:(q + 1) * 512], in_=acc[q])

                    # ---- chain steps ----
                    for k in (1, 2, 3):
                        pat = pats[k - 1]
                        # op1: replicated v patterns (K=128 selection folds top+bottom)
                        op1 = ops1.tile([128, N_S, FD], dt.bfloat16, tag="op1",
                                        name=f"op1_{rep}_{k}")
                        for s in range(N_S):
                            for hh in range(NQ // 2):
                                p1 = ps_op.tile([128, 1024], dt.float32, tag="op",
                                                name=f"op1p_{rep}_{k}_{s}_{hh}")
                                for q in range(2):
                                    c0 = hh * 1024 + q * 512
                                    nc.tensor.matmul(p1[:, q * 512:(q + 1) * 512],
                                                     r1_sb[:, s, :],
                                                     vT[:, c0:c0 + 512],
                                                     start=True, stop=True)
                                nc.scalar.copy(out=op1[:, s, hh * 1024:(hh + 1) * 1024],
                                               in_=p1)
                        # z chunks + accumulation (even chunks -> top, odd -> bottom)
                        acc = [ps_acc.tile([128, 512], dt.float32, tag=f"acc{q}",
                                           name=f"acc_{rep}_{k}_{q}") for q in range(NQ)]
                        for c in range(N_CHUNK):
                            s, t = c // N_T, c % N_T
                            z = zp.tile([128, FD], dt.bfloat16, tag="z", name=f"z_{rep}_{k}_{c}")
                            if GP_CHUNK_MOD and c % GP_CHUNK_MOD == 2:
                                nc.gpsimd.tensor_mul(z, op1[:, s, :], pat[:, t, :])
                            else:
                                nc.vector.tensor_mul(z, op1[:, s, :], pat[:, t, :])
                            half = c % 2
                            tp = (0, 64 * half)
                            for q in range(NQ):
                                nc.tensor.matmul(acc[q][64 * half:64 * (half + 1), :],
                                                 lhst_sb[:, k - 1, c, :],
                                                 z[:, q * 512:(q + 1) * 512],
                                                 start=(c < 2), stop=(c >= N_CHUNK - 2),
                                                 tile_position=tp)
                        if k < 3:
                            vT = vp.tile([128, FD], dt.bfloat16, tag="v", name=f"v{rep}_{k}")
                            for q in range(NQ):
                                nc.scalar.copy(out=vT[:, q * 512:(q + 1) * 512], in_=acc[q])
                        else:
                            vtop = outp.tile([O, FD], dt.float32, tag="vtop", name=f"vtop_{rep}")
                            outT = outp.tile([O, FD], dt.float32, tag="outT", name=f"outT_{rep}")
                            for q in range(NQ):
                                sl = slice(q * 512, (q + 1) * 512)
                                nc.scalar.copy(out=vtop[:, sl], in_=acc[q][0:O, :])
                                nc.vector.tensor_add(outT[:, sl], vtop[:, sl],
                                                     acc[q][O:128, :])
                            for b in range(IPC):
                                nc.sync.dma_start(out=out_d[b],
                                                  in_=outT[:, b * BLK:b * BLK + NIMG])

    nc.compile()
    return nc


def _build_weights(tensors):
    T = np.asarray(tensors, dtype=np.float32)  # (4, O, O, C): [w, i, j, p]
    a0 = np.zeros((C, 128), dtype=BF16)
    a0[:, :O] = T[0, 0].T.astype(BF16)                           # (p, j)
    lhst = np.zeros((3, N_CHUNK, 128, O), dtype=BF16)
    for k in range(1, 4):
        t_ipj = np.ascontiguousarray(T[k].transpose(0, 2, 1))    # (i, p, j)
        for s in range(N_S):
            for t in range(N_T):
                blk = t_ipj[s * A_SET:(s + 1) * A_SET,
                            t * B_SET:(t + 1) * B_SET, :]
                lhst[k - 1, s * N_T + t] = blk.reshape(128, O).astype(BF16)
    r1 = np.zeros((N_S, 128, 128), dtype=BF16)
    for s in range(N_S):
        for lane in range(128):
            i = s * A_SET + lane // B_SET
            r1[s, i, lane] = 1.0
            r1[s, O + i, lane] = 1.0
    r2 = np.zeros((N_T, C, 128), dtype=BF16)
    for t in range(N_T):
        for lane in range(128):
            r2[t, t * B_SET + lane % B_SET, lane] = 1.0
    return {"lhst": lhst, "a0": a0, "r1": r1, "r2": r2}


_CACHE = {}


def _get_program(reps=1, loop_n=1):
    key = f"nc{reps}_{loop_n}_{GP_CHUNK_MOD}"
    if key not in _CACHE:
        _CACHE[key] = _build_program(reps, loop_n)
    return _CACHE[key]


def run(input_data, tensors, trace=False, reps=1, loop_n=1):
    nc = _get_program(reps, loop_n)
    w = _build_weights(tensors)
    x16 = np.asarray(input_data, dtype=np.float32).astype(BF16)
    in_maps = []
    for c in range(NCORES):
        m = dict(w)
        m["x"] = np.ascontiguousarray(x16[c * IPC:(c + 1) * IPC])
        in_maps.append(m)
    res = bass_utils.run_bass_kernel_spmd(nc, in_maps, core_ids=list(range(NCORES)),
                                          trace=trace)
    outs = np.concatenate([res.results[c]["out"] for c in range(NCORES)], axis=0)
    out = outs.reshape(B, O, OH, OW).astype(np.float32)
    return out, res


def kernel(input_data, tensors):
    out, _ = run(input_data, tensors)
    return out



# revision 4
# speedup vs baseline: 1.0844x; 1.0844x over previous
"""Trainium2 Bass kernel for nn_Conv2dAMPS.

Reference computation: im2col with a 2x2 kernel (4 positions), per-sample
matrices M_w = tensors[w] . emb_w (contract channels), output = row 0 of
M_0 @ M_1 @ M_2 @ M_3, reshaped to (B, O, oh, ow).

Only row 0 of the matrix product is needed, so the chain collapses to a
vector-matrix chain per sample.  Additionally the first two links are
merged into one bilinear stage via the precomputed
    U[(p0,p1), j] = sum_i T0[0,i,p0] * T1[i,j,p1]
so the whole computation is three kron-contraction stages:
    A:  v1 = kron(e0, e1) @ U
    B:  v2 = kron(v1, e2) @ W2r      (W2r[(i,p),j] = T2[i,j,p])
    C:  out = kron(v2, e3) @ W3r
Each stage: the per-sample outer product z = a (x) b is built on the
vector engine (plus a few decoupled chunks on gpsimd) from
partition-replicated operand tiles produced by 0/1-selection matmuls on
the tensor engine; z chunks (8 a's x 16 b's = 128 lanes) feed K=128
matmuls accumulating into PSUM with 2x column tiling (even/odd chunks in
top/bottom array halves), and the top+bottom fold is absorbed into the
next stage's replication matmul (K=128 selection).

Stage A has no dependency on previous-stage results, so its operand
replication can begin as soon as the (host-side packed, contiguous)
im2col loads land -- there is no serial "v0 + pats" head.  pats for
stage k+1 are built during stage k.

im2col (host-side packing): e0ext = x[:, 0:32, 0:31] (n = h*31+w),
e1ext = x[:, 0:32, 1:32]; e2 = e0ext[n+31], e3 = e1ext[n+31], so the
kernel only ever slices one SBUF-resident pair of extended images.

Sharding: data-parallel over batch B (2 images per core, 8 cores),
weights replicated.
"""

import sys

sys.path.insert(0, "/opt/trn_rl_repo")

import numpy as np
import ml_dtypes

import concourse.bacc as bacc
import concourse.mybir as mybir
import concourse.tile as tile
from concourse import bass_utils

BF16 = ml_dtypes.bfloat16

B, C, H, W = 16, 64, 32, 32
O = 64
OH = OW = 31
NIMG = OH * OW            # 961 real samples per image
NEXT = 992                # extended im2col columns (32 rows x 31)
BLK = 1024                # column block per image
NCORES = 8
IPC = B // NCORES         # images per core
FD = IPC * BLK            # 2048 free columns per core
NQ = FD // 512            # psum quarters

A_SET = 8                 # left-factor values per chunk-set (replicated 16x)
B_SET = 16                # right-factor values per chunk-set (replicated 8x)
N_S = O // A_SET          # 8
N_T = O // B_SET          # 4
N_CHUNK = N_S * N_T       # 32

# (right src slot, right shift) per stage; the left factor of stage A is
# e0 (slot 0 shift 0); stages B/C take the left factor from the previous
# stage's v.
STAGE_RIGHT = {0: (1, 0), 1: (0, 31), 2: (1, 31)}

# chunks (by (s,t) index c = s*N_T+t) assigned to gpsimd per stage.
# gpsimd TT is ~4x slower than DVE; give it a few chunks that are
# consumed late in the accumulation order so its latency stays hidden.
GP_CHUNKS = {
    0: (7, 11, 15, 19, 23, 27),
    1: (7, 11, 15, 19, 23),
    2: (7, 11, 15, 19, 23),
}
# number of DVE chunks kept after the last gpsimd chunk in the
# accumulation order (so PE never stalls on a late gpsimd TT).
N_DVE_TAIL = 4


def _chunk_order(stage):
    gp = GP_CHUNKS[stage]
    dve = [c for c in range(N_CHUNK) if c not in gp]
    head = dve[: len(dve) - N_DVE_TAIL]
    tail = dve[len(dve) - N_DVE_TAIL:]
    order = head + list(gp) + tail
    return order, set(gp)


def _build_program(reps=1, loop_n=1):
    nc = bacc.Bacc("TRN2", target_bir_lowering=False, debug=False)
    dt = mybir.dt

    # host-packed extended im2col: (IPC, C, 2, BLK) -- e0ext / e1ext
    xe_d = nc.dram_tensor("xe", [IPC, C, 2, BLK], dt.bfloat16,
                          kind="ExternalInput").ap()
    # stage weights: [3, N_CHUNK, 128, O]  (stage A = U, stages B/C = T2/T3)
    lhst_d = nc.dram_tensor("lhst", [3, N_CHUNK, 128, O], dt.bfloat16,
                            kind="ExternalInput").ap()
    # stage-A left replication (K=64, no fold)
    ra_d = nc.dram_tensor("ra", [N_S, C, 128], dt.bfloat16,
                          kind="ExternalInput").ap()
    # stage-B/C left replication (K=128, folds top+bottom acc halves)
    r1_d = nc.dram_tensor("r1", [N_S, 128, 128], dt.bfloat16,
                          kind="ExternalInput").ap()
    # right replication (K=64)
    r2_d = nc.dram_tensor("r2", [N_T, C, 128], dt.bfloat16,
                          kind="ExternalInput").ap()
    out_d = nc.dram_tensor("out", [IPC, O, NIMG], dt.float32,
                           kind="ExternalOutput").ap()

    with tile.TileContext(nc) as tc:
        with (
            tc.tile_pool(name="consts", bufs=1) as consts,
            tc.tile_pool(name="embp", bufs=1) as embp,
            tc.tile_pool(name="patp", bufs=2) as patp,
            tc.tile_pool(name="ops1", bufs=2) as ops1,
            tc.tile_pool(name="zp", bufs=6) as zp,
            tc.tile_pool(name="vp", bufs=2) as vp,
            tc.tile_pool(name="outp", bufs=1) as outp,
            tc.tile_pool(name="ps_op", bufs=2, space="PSUM") as ps_op,
            tc.tile_pool(name="ps_acc", bufs=1, space="PSUM") as ps_acc,
        ):
            # ---- constants: small selection matrices first, then the big
            # stage weights (so replication matmuls can start early and
            # keep the PE warm while lhst streams in).
            ra_sb = consts.tile([C, N_S, 128], dt.bfloat16)
            nc.sync.dma_start(out=ra_sb, in_=ra_d.rearrange("s k l -> k s l"))
            r2_sb = consts.tile([C, N_T, 128], dt.bfloat16)
            nc.sync.dma_start(out=r2_sb, in_=r2_d.rearrange("t k l -> k t l"))
            r1_sb = consts.tile([128, N_S, 128], dt.bfloat16)
            nc.sync.dma_start(out=r1_sb, in_=r1_d.rearrange("s k l -> k s l"))
            lhst_sb = consts.tile([128, 3, N_CHUNK, O], dt.bfloat16)
            nc.sync.dma_start(out=lhst_sb, in_=lhst_d.rearrange("k c l j -> l k c j"))

            loop_cm = tc.For_i(0, loop_n, 1) if loop_n > 1 else None
            import contextlib
            with (loop_cm if loop_cm is not None else contextlib.nullcontext()):
                for rep in range(reps):
                    # ---- extended im2col loads (contiguous, host-packed)
                    embT = embp.tile([C, 2, FD + 32], dt.bfloat16)
                    nc.vector.memset(embT[:, :, FD:FD + 32], 0.0)
                    for b in range(IPC):
                        for e in range(2):
                            nc.sync.dma_start(
                                out=embT[:, e, b * BLK:(b + 1) * BLK],
                                in_=xe_d[b, :, e, :])

                    def right_pat(stage, tag):
                        """replicated right factor (4 t-slices) for stage."""
                        e, off = STAGE_RIGHT[stage]
                        pat = patp.tile([128, N_T, FD], dt.bfloat16, tag="pat",
                                        name=f"pat{rep}_{stage}")
                        for t in range(N_T):
                            for hh in range(NQ // 2):
                                p2 = ps_op.tile([128, 1024], dt.float32, tag="op",
                                                name=f"patp_{rep}_{stage}_{t}_{hh}")
                                for q in range(2):
                                    c0 = hh * 1024 + q * 512
                                    nc.tensor.matmul(p2[:, q * 512:(q + 1) * 512],
                                                     r2_sb[:, t, :],
                                                     embT[:, e, c0 + off:c0 + off + 512],
                                                     start=True, stop=True)
                                nc.scalar.copy(out=pat[:, t, hh * 1024:(hh + 1) * 1024],
                                               in_=p2)
                        return pat

                    def left_rep(stage, vT, tag):
                        """replicated left factor (8 s-slices).

                        stage A: K=64 selection from e0 (embT slot 0);
                        stages B/C: K=128 fold+selection from vT."""
                        op1 = ops1.tile([128, N_S, FD], dt.bfloat16, tag="op1",
                                        name=f"op1_{rep}_{stage}")
                        for s in range(N_S):
                            for hh in range(NQ // 2):
                                p1 = ps_op.tile([128, 1024], dt.float32, tag="op",
                                                name=f"op1p_{rep}_{stage}_{s}_{hh}")
                                for q in range(2):
                                    c0 = hh * 1024 + q * 512
                                    if stage == 0:
                                        nc.tensor.matmul(p1[:, q * 512:(q + 1) * 512],
                                                         ra_sb[:, s, :],
                                                         embT[:, 0, c0:c0 + 512],
                                                         start=True, stop=True)
                                    else:
                                        nc.tensor.matmul(p1[:, q * 512:(q + 1) * 512],
                                                         r1_sb[:, s, :],
                                                         vT[:, c0:c0 + 512],
                                                         start=True, stop=True)
                                nc.scalar.copy(out=op1[:, s, hh * 1024:(hh + 1) * 1024],
                                               in_=p1)
                        return op1

                    # ---- stage A operands (no step dependency)
                    op1 = left_rep(0, None, tag="op1A")
                    pat = right_pat(0, tag="patA")

                    vT = None
                    for stage in range(3):
                        order, gp_set = _chunk_order(stage)
                        acc = [ps_acc.tile([128, 512], dt.float32, tag=f"acc{q}",
                                           name=f"acc_{rep}_{stage}_{q}")
                               for q in range(NQ)]
                        # issue gpsimd TTs first so they start as soon as
                        # their operand slices are ready
                        zmap = {}
                        for c in sorted(gp_set):
                            s, t = c // N_T, c % N_T
                            z = zp.tile([128, FD], dt.bfloat16, tag="zg",
                                        name=f"zg_{rep}_{stage}_{c}")
                            nc.gpsimd.tensor_mul(z, op1[:, s, :], pat[:, t, :])
                            zmap[c] = z
                        pat_next = None
                        for idx, c in enumerate(order):
                            s, t = c // N_T, c % N_T
                            if c in gp_set:
                                z = zmap[c]
                            else:
                                z = zp.tile([128, FD], dt.bfloat16, tag="z",
                                            name=f"z_{rep}_{stage}_{c}")
                                nc.vector.tensor_mul(z, op1[:, s, :], pat[:, t, :])
                            half = idx % 2
                            tp = (0, 64 * half)
                            for q in range(NQ):
                                nc.tensor.matmul(acc[q][64 * half:64 * (half + 1), :],
                                                 lhst_sb[:, stage, c, :],
                                                 z[:, q * 512:(q + 1) * 512],
                                                 start=(idx < 2),
                                                 stop=(idx >= N_CHUNK - 2),
                                                 tile_position=tp)
                            # build next stage's right factor mid-stage
                            # (PE/ACT have spare capacity while DVE streams)
                            if idx == 2 and stage < 2:
                                pat_next = right_pat(
                                    stage + 1, tag="patA" if stage == 1 else "patB")
                        if stage < 2:
                            vT = vp.tile([128, FD], dt.bfloat16, tag="v",
                                         name=f"v{rep}_{stage}")
                            for q in range(NQ):
                                nc.scalar.copy(out=vT[:, q * 512:(q + 1) * 512],
                                               in_=acc[q])
                            op1 = left_rep(stage + 1, vT,
                                           tag="op1B" if stage == 0 else "op1A")
                            pat = pat_next
                        else:
                            vtop = outp.tile([O, FD], dt.float32, tag="vtop",
                                             name=f"vtop_{rep}")
                            outT = outp.tile([O, FD], dt.float32, tag="outT",
                                             name=f"outT_{rep}")
                            for q in range(NQ):
                                sl = slice(q * 512, (q + 1) * 512)
                                nc.scalar.copy(out=vtop[:, sl], in_=acc[q][0:O, :])
                                nc.vector.tensor_add(outT[:, sl], vtop[:, sl],
                                                     acc[q][O:128, :])
                            for b in range(IPC):
                                nc.sync.dma_start(out=out_d[b],
                                                  in_=outT[:, b * BLK:b * BLK + NIMG])

    nc.compile()
    return nc


def _build_weights(tensors):
    T = np.asarray(tensors, dtype=np.float32)  # (4, O, O, C): [w, i, j, p]
    lhst = np.zeros((3, N_CHUNK, 128, O), dtype=BF16)
    # stage A: U[(p0,p1), j] = sum_i T0[0,i,p0] * T1[i,j,p1]
    U = np.einsum('ip,ijq->pqj', T[0, 0], T[1])          # (p0, p1, j)
    for s in range(N_S):
        for t in range(N_T):
            blk = U[s * A_SET:(s + 1) * A_SET,
                    t * B_SET:(t + 1) * B_SET, :]
            lhst[0, s * N_T + t] = blk.reshape(128, O).astype(BF16)
    # stages B/C: W[(i,p), j] = T[k][i, j, p]
    for k in (2, 3):
        t_ipj = np.ascontiguousarray(T[k].transpose(0, 2, 1))    # (i, p, j)
        for s in range(N_S):
            for t in range(N_T):
                blk = t_ipj[s * A_SET:(s + 1) * A_SET,
                            t * B_SET:(t + 1) * B_SET, :]
                lhst[k - 1, s * N_T + t] = blk.reshape(128, O).astype(BF16)
    # stage-A left selection: lane <- e0 row s*8 + lane//16  (K=64)
    ra = np.zeros((N_S, C, 128), dtype=BF16)
    for s in range(N_S):
        for lane in range(128):
            ra[s, s * A_SET + lane // B_SET, lane] = 1.0
    # stage-B/C left selection with fold (K=128)
    r1 = np.zeros((N_S, 128, 128), dtype=BF16)
    for s in range(N_S):
        for lane in range(128):
            i = s * A_SET + lane // B_SET
            r1[s, i, lane] = 1.0
            r1[s, O + i, lane] = 1.0
    # right selection: lane <- row t*16 + lane%16  (K=64)
    r2 = np.zeros((N_T, C, 128), dtype=BF16)
    for t in range(N_T):
        for lane in range(128):
            r2[t, t * B_SET + lane % B_SET, lane] = 1.0
    return {"lhst": lhst, "ra": ra, "r1": r1, "r2": r2}


def _pack_inputs(input_data):
    """host-side im2col packing: (B, C, 2, BLK) bf16, n = h*31+w."""
    x = np.asarray(input_data, dtype=np.float32)
    xe = np.zeros((B, C, 2, BLK), dtype=BF16)
    xe[:, :, 0, :NEXT] = x[:, :, :, 0:31].reshape(B, C, NEXT).astype(BF16)
    xe[:, :, 1, :NEXT] = x[:, :, :, 1:32].reshape(B, C, NEXT).astype(BF16)
    return xe


_CACHE = {}


def _get_program(reps=1, loop_n=1):
    key = f"nc{reps}_{loop_n}"
    if key not in _CACHE:
        _CACHE[key] = _build_program(reps, loop_n)
    return _CACHE[key]


def run(input_data, tensors, trace=False, reps=1, loop_n=1):
    nc = _get_program(reps, loop_n)
    w = _build_weights(tensors)
    xe = _pack_inputs(input_data)
    in_maps = []
    for c in range(NCORES):
        m = dict(w)
        m["xe"] = np.ascontiguousarray(xe[c * IPC:(c + 1) * IPC])
        in_maps.append(m)
    res = bass_utils.run_bass_kernel_spmd(nc, in_maps, core_ids=list(range(NCORES)),
                                          trace=trace)
    outs = np.concatenate([res.results[c]["out"] for c in range(NCORES)], axis=0)
    out = outs.reshape(B, O, OH, OW).astype(np.float32)
    return out, res


def kernel(input_data, tensors):
    out, _ = run(input_data, tensors)
    return out


# revision 7
# speedup vs baseline: 1.3702x; 1.2636x over previous
"""Trainium2 Bass kernel for nn_Conv2dAMPS.

Reference computation: im2col with a 2x2 kernel (4 positions), per-sample
matrices M_w = tensors[w] . emb_w (contract channels), output = row 0 of
M_0 @ M_1 @ M_2 @ M_3, reshaped to (B, O, oh, ow).

Only row 0 of the matrix product is needed, so the chain collapses to a
vector-matrix chain per sample.  Additionally the first two links are
merged into one bilinear stage via the precomputed
    U[(p0,p1), j] = sum_i T0[0,i,p0] * T1[i,j,p1]
so the whole computation is three kron-contraction stages:
    A:  v1 = kron(e0, e1) @ U
    B:  v2 = kron(v1, e2) @ W2r      (W2r[(i,p),j] = T2[i,j,p])
    C:  out = kron(v2, e3) @ W3r
Each stage: the per-sample outer product z = a (x) b is built on the
vector engine (plus a few decoupled chunks on gpsimd) from
partition-replicated operand tiles produced by 0/1-selection matmuls on
the tensor engine; z chunks (8 a's x 16 b's = 128 lanes) feed K=128
matmuls accumulating into PSUM with 2x column tiling (even/odd chunks in
top/bottom array halves), and the top+bottom fold is absorbed into the
next stage's replication matmul (K=128 selection).

Stage A has no dependency on previous-stage results, so its operand
replication can begin as soon as the (host-side packed, contiguous)
im2col loads land -- there is no serial "v0 + pats" head.  pats for
stage k+1 are built during stage k.

im2col (host-side packing): e0ext = x[:, 0:32, 0:31] (n = h*31+w),
e1ext = x[:, 0:32, 1:32]; e2 = e0ext[n+31], e3 = e1ext[n+31], so the
kernel only ever slices one SBUF-resident pair of extended images.

Sharding: data-parallel over batch B (2 images per core, 8 cores),
weights replicated.
"""

import sys

sys.path.insert(0, "/opt/trn_rl_repo")

import numpy as np
import ml_dtypes

import concourse.bacc as bacc
import concourse.mybir as mybir
import concourse.tile as tile
from concourse import bass_utils

BF16 = ml_dtypes.bfloat16

B, C, H, W = 16, 64, 32, 32
O = 64
OH = OW = 31
NIMG = OH * OW            # 961 real samples per image
NEXT = 992                # extended im2col columns (32 rows x 31)
BLK = 1024                # column block per image
NCORES = 8
IPC = B // NCORES         # images per core
FD = IPC * BLK            # 2048 free columns per core
NQ = FD // 512            # psum quarters

A_SET = 8                 # left-factor values per chunk-set (replicated 16x)
B_SET = 16                # right-factor values per chunk-set (replicated 8x)
N_S = O // A_SET          # 8
N_T = O // B_SET          # 4
N_CHUNK = N_S * N_T       # 32

# (right src slot, right shift) per stage; the left factor of stage A is
# e0 (slot 0 shift 0); stages B/C take the left factor from the previous
# stage's v.
STAGE_RIGHT = {0: (1, 0), 1: (0, 31), 2: (1, 31)}

# chunks (by (s,t) index c = s*N_T+t) assigned to gpsimd per stage.
# gpsimd TT is ~4x slower than DVE; give it a few chunks that are
# consumed late in the accumulation order so its latency stays hidden.
GP_CHUNKS = {0: (), 1: (), 2: ()}
# number of DVE chunks kept after the last gpsimd chunk in the
# accumulation order (so PE never stalls on a late gpsimd TT).
N_DVE_TAIL = 4


def _chunk_order(stage):
    gp = GP_CHUNKS[stage]
    dve = [c for c in range(N_CHUNK) if c not in gp]
    head = dve[: len(dve) - N_DVE_TAIL]
    tail = dve[len(dve) - N_DVE_TAIL:]
    order = head + list(gp) + tail
    return order, set(gp)


def _build_program(reps=1, loop_n=1):
    nc = bacc.Bacc("TRN2", target_bir_lowering=False, debug=False)
    dt = mybir.dt

    # host-packed extended im2col: (IPC, C, 2, BLK) -- e0ext / e1ext
    xe_d = nc.dram_tensor("xe", [IPC, C, 2, BLK], dt.bfloat16,
                          kind="ExternalInput").ap()
    # stage weights: [3, N_CHUNK, 128, O]  (stage A = U, stages B/C = T2/T3)
    lhst_d = nc.dram_tensor("lhst", [128, 3, N_CHUNK, O], dt.bfloat16,
                            kind="ExternalInput").ap()
    # stage-A left replication (K=64, no fold); partition-major layouts
    ra_d = nc.dram_tensor("ra", [C, N_S, 128], dt.bfloat16,
                          kind="ExternalInput").ap()
    # stage-B/C left replication (K=128, folds top+bottom acc halves)
    r1_d = nc.dram_tensor("r1", [128, N_S, 128], dt.bfloat16,
                          kind="ExternalInput").ap()
    # right replication (K=64)
    r2_d = nc.dram_tensor("r2", [C, N_T, 128], dt.bfloat16,
                          kind="ExternalInput").ap()
    out_d = nc.dram_tensor("out", [IPC, O, NIMG], dt.float32,
                           kind="ExternalOutput").ap()

    with tile.TileContext(nc) as tc:
        with (
            tc.tile_pool(name="consts", bufs=1) as consts,
            tc.tile_pool(name="embp", bufs=1) as embp,
            tc.tile_pool(name="patp", bufs=2) as patp,
            tc.tile_pool(name="ops1", bufs=2) as ops1,
            tc.tile_pool(name="zp", bufs=6) as zp,
            tc.tile_pool(name="vp", bufs=2) as vp,
            tc.tile_pool(name="outp", bufs=1) as outp,
            tc.tile_pool(name="ps_op", bufs=2, space="PSUM") as ps_op,
            tc.tile_pool(name="ps_acc", bufs=1, space="PSUM") as ps_acc,
        ):
            # ---- constants: small selection matrices first, then the big
            # stage weights (so replication matmuls can start early and
            # keep the PE warm while lhst streams in).
            ra_sb = consts.tile([C, N_S, 128], dt.bfloat16)
            nc.sync.dma_start(out=ra_sb, in_=ra_d)
            r2_sb = consts.tile([C, N_T, 128], dt.bfloat16)
            nc.sync.dma_start(out=r2_sb, in_=r2_d)
            r1_sb = consts.tile([128, N_S, 128], dt.bfloat16)
            nc.sync.dma_start(out=r1_sb, in_=r1_d)
            lhst_sb = consts.tile([128, 3, N_CHUNK, O], dt.bfloat16)
            nc.sync.dma_start(out=lhst_sb, in_=lhst_d)

            loop_cm = tc.For_i(0, loop_n, 1) if loop_n > 1 else None
            import contextlib
            with (loop_cm if loop_cm is not None else contextlib.nullcontext()):
                for rep in range(reps):
                    # ---- extended im2col loads (contiguous, host-packed)
                    embT = embp.tile([C, 2, FD + 32], dt.bfloat16)
                    nc.vector.memset(embT[:, :, FD:FD + 32], 0.0)
                    for b in range(IPC):
                        for e in range(2):
                            nc.sync.dma_start(
                                out=embT[:, e, b * BLK:(b + 1) * BLK],
                                in_=xe_d[b, :, e, :])

                    def pat_slice(pat, stage, t):
                        """build one replicated right-factor t-slice."""
                        e, off = STAGE_RIGHT[stage]
                        for hh in range(NQ // 2):
                            p2 = ps_op.tile([128, 1024], dt.float32, tag="op",
                                            name=f"patp_{rep}_{stage}_{t}_{hh}")
                            for q in range(2):
                                c0 = hh * 1024 + q * 512
                                nc.tensor.matmul(p2[:, q * 512:(q + 1) * 512],
                                                 r2_sb[:, t, :],
                                                 embT[:, e, c0 + off:c0 + off + 512],
                                                 start=True, stop=True)
                            nc.scalar.copy(out=pat[:, t, hh * 1024:(hh + 1) * 1024],
                                           in_=p2)

                    def op1_slice(op1, stage, s, vT):
                        """build one replicated left-factor s-slice.

                        stage A: K=64 selection from e0 (embT slot 0);
                        stages B/C: K=128 fold+selection from vT."""
                        for hh in range(NQ // 2):
                            p1 = ps_op.tile([128, 1024], dt.float32, tag="op",
                                            name=f"op1p_{rep}_{stage}_{s}_{hh}")
                            for q in range(2):
                                c0 = hh * 1024 + q * 512
                                if stage == 0:
                                    nc.tensor.matmul(p1[:, q * 512:(q + 1) * 512],
                                                     ra_sb[:, s, :],
                                                     embT[:, 0, c0:c0 + 512],
                                                     start=True, stop=True)
                                else:
                                    nc.tensor.matmul(p1[:, q * 512:(q + 1) * 512],
                                                     r1_sb[:, s, :],
                                                     vT[:, c0:c0 + 512],
                                                     start=True, stop=True)
                            nc.scalar.copy(out=op1[:, s, hh * 1024:(hh + 1) * 1024],
                                           in_=p1)

                    # ---- PE warmup: dense tiny matmuls flip the HAM clock
                    # gate to 8/8 while DMAs stream; the result is dumped
                    # into embT's zero tail (read only by junk columns).
                    if rep == 0:
                        wps = ps_op.tile([128, 512], dt.float32, tag="op",
                                         name="warm_ps")
                        for w in range(48):
                            nc.tensor.matmul(wps[:, (w % 4) * 128:(w % 4) * 128 + 128],
                                             ra_sb[:, w % N_S, :],
                                             ra_sb[:, (w + 1) % N_S, :],
                                             start=True, stop=True)
                        nc.scalar.copy(out=embT[0:C, 1, FD + 16:FD + 32],
                                       in_=wps[0:C, 0:16])

                    # ---- stage A operands (no step dependency); interleave
                    # s/t slice builds so early chunks' operands finish first
                    op1 = ops1.tile([128, N_S, FD], dt.bfloat16, tag="op1",
                                    name=f"op1_{rep}_0")
                    pat = patp.tile([128, N_T, FD], dt.bfloat16, tag="pat",
                                    name=f"pat{rep}_0")
                    op1_slice(op1, 0, 0, None)
                    pat_slice(pat, 0, 0)
                    pat_slice(pat, 0, 1)
                    op1_slice(op1, 0, 1, None)
                    pat_slice(pat, 0, 2)
                    pat_slice(pat, 0, 3)
                    for s in range(2, N_S):
                        op1_slice(op1, 0, s, None)

                    vT = None
                    for stage in range(3):
                        order, gp_set = _chunk_order(stage)
                        acc = [ps_acc.tile([128, 512], dt.float32, tag=f"acc{q}",
                                           name=f"acc_{rep}_{stage}_{q}")
                               for q in range(NQ)]
                        pat_next = None
                        for idx, c in enumerate(order):
                            s, t = c // N_T, c % N_T
                            z = zp.tile([128, FD], dt.bfloat16, tag="z",
                                        name=f"z_{rep}_{stage}_{c}")
                            nc.vector.tensor_mul(z, op1[:, s, :], pat[:, t, :])
                            half = idx % 2
                            tp = (0, 64 * half)
                            for q in range(NQ):
                                nc.tensor.matmul(acc[q][64 * half:64 * (half + 1), :],
                                                 lhst_sb[:, stage, c, :],
                                                 z[:, q * 512:(q + 1) * 512],
                                                 start=(idx < 2),
                                                 stop=(idx >= N_CHUNK - 2),
                                                 tile_position=tp)
                            # build next stage's right factor mid-stage
                            # (PE/ACT have spare capacity while DVE streams)
                            if stage < 2 and idx in (4, 8, 12, 16):
                                if idx == 4:
                                    pat_next = patp.tile(
                                        [128, N_T, FD], dt.bfloat16, tag="pat",
                                        name=f"pat{rep}_{stage + 1}")
                                pat_slice(pat_next, stage + 1, idx // 4 - 1)
                        if stage < 2:
                            vT = vp.tile([128, FD], dt.bfloat16, tag="v",
                                         name=f"v{rep}_{stage}")
                            op1n = ops1.tile([128, N_S, FD], dt.bfloat16, tag="op1",
                                             name=f"op1_{rep}_{stage + 1}")
                            for q in range(NQ):
                                nc.scalar.copy(out=vT[:, q * 512:(q + 1) * 512],
                                               in_=acc[q])
                            for s in range(N_S):
                                op1_slice(op1n, stage + 1, s, vT)
                            op1 = op1n
                            pat = pat_next
                        else:
                            vtop = outp.tile([O, FD], dt.float32, tag="vtop",
                                             name=f"vtop_{rep}")
                            outT = outp.tile([O, FD], dt.float32, tag="outT",
                                             name=f"outT_{rep}")
                            for q in range(NQ):
                                sl = slice(q * 512, (q + 1) * 512)
                                nc.scalar.copy(out=vtop[:, sl], in_=acc[q][0:O, :])
                                nc.vector.tensor_add(outT[:, sl], vtop[:, sl],
                                                     acc[q][O:128, :])
                            for b in range(IPC):
                                nc.sync.dma_start(out=out_d[b],
                                                  in_=outT[:, b * BLK:b * BLK + NIMG])

    nc.compile()
    return nc


def _build_weights(tensors):
    T = np.asarray(tensors, dtype=np.float32)  # (4, O, O, C): [w, i, j, p]
    lhst = np.zeros((3, N_CHUNK, 128, O), dtype=BF16)
    # stage A: U[(p0,p1), j] = sum_i T0[0,i,p0] * T1[i,j,p1]
    U = np.einsum('ip,ijq->pqj', T[0, 0], T[1])          # (p0, p1, j)
    for s in range(N_S):
        for t in range(N_T):
            blk = U[s * A_SET:(s + 1) * A_SET,
                    t * B_SET:(t + 1) * B_SET, :]
            lhst[0, s * N_T + t] = blk.reshape(128, O).astype(BF16)
    # stages B/C: W[(i,p), j] = T[k][i, j, p]
    for k in (2, 3):
        t_ipj = np.ascontiguousarray(T[k].transpose(0, 2, 1))    # (i, p, j)
        for s in range(N_S):
            for t in range(N_T):
                blk = t_ipj[s * A_SET:(s + 1) * A_SET,
                            t * B_SET:(t + 1) * B_SET, :]
                lhst[k - 1, s * N_T + t] = blk.reshape(128, O).astype(BF16)
    # stage-A left selection: lane <- e0 row s*8 + lane//16  (K=64)
    ra = np.zeros((C, N_S, 128), dtype=BF16)
    for s in range(N_S):
        for lane in range(128):
            ra[s * A_SET + lane // B_SET, s, lane] = 1.0
    # stage-B/C left selection with fold (K=128)
    r1 = np.zeros((128, N_S, 128), dtype=BF16)
    for s in range(N_S):
        for lane in range(128):
            i = s * A_SET + lane // B_SET
            r1[i, s, lane] = 1.0
            r1[O + i, s, lane] = 1.0
    # right selection: lane <- row t*16 + lane%16  (K=64)
    r2 = np.zeros((C, N_T, 128), dtype=BF16)
    for t in range(N_T):
        for lane in range(128):
            r2[t * B_SET + lane % B_SET, t, lane] = 1.0
    # lhst partition-major: (128, 3, N_CHUNK, O)
    lhst = np.ascontiguousarray(lhst.transpose(2, 0, 1, 3))
    return {"lhst": lhst, "ra": ra, "r1": r1, "r2": r2}


def _pack_inputs(input_data):
    """host-side im2col packing: (B, C, 2, BLK) bf16, n = h*31+w."""
    x = np.asarray(input_data, dtype=np.float32)
    xe = np.zeros((B, C, 2, BLK), dtype=BF16)
    xe[:, :, 0, :NEXT] = x[:, :, :, 0:31].reshape(B, C, NEXT).astype(BF16)
    xe[:, :, 1, :NEXT] = x[:, :, :, 1:32].reshape(B, C, NEXT).astype(BF16)
    return xe


_CACHE = {}


def _get_program(reps=1, loop_n=1):
    key = f"nc{reps}_{loop_n}"
    if key not in _CACHE:
        _CACHE[key] = _build_program(reps, loop_n)
    return _CACHE[key]


def run(input_data, tensors, trace=False, reps=1, loop_n=1):
    nc = _get_program(reps, loop_n)
    w = _build_weights(tensors)
    xe = _pack_inputs(input_data)
    in_maps = []
    for c in range(NCORES):
        m = dict(w)
        m["xe"] = np.ascontiguousarray(xe[c * IPC:(c + 1) * IPC])
        in_maps.append(m)
    res = bass_utils.run_bass_kernel_spmd(nc, in_maps, core_ids=list(range(NCORES)),
                                          trace=trace)
    outs = np.concatenate([res.results[c]["out"] for c in range(NCORES)], axis=0)
    out = outs.reshape(B, O, OH, OW).astype(np.float32)
    return out, res


def kernel(input_data, tensors):
    out, _ = run(input_data, tensors)
    return out
